# revision 63
# baseline (speedup 1.0000x reference)
"""Multi-head causal attention (B=2, S=2048, D=1024, H=16, DH=64) on 8 NeuronCores.

Sharding: data-parallel over batch (2) x tensor-parallel over heads (4 groups
of 4 heads). Core c handles batch c//4, heads 4*(c%4)..4*(c%4)+3. Each core
computes its head-group's Q/K/V projections, causal softmax attention, and a
partial output projection (Wo row-shard); the host sums the 4 partials per
batch.

Device-side layout choices:
- Inputs are uploaded transposed (xT: [D, S]) and in bf16 so every projection
  matmul contracts over the partition dim with contiguous DMA.
- q/k are produced transposed ([dh, s]); scores are computed transposed
  ([k, q]) so the softmax sum folds into the attention*V matmul via an extra
  ones-column on V, and the context comes out as ctxT [dh, q], which is
  exactly the stationary operand the Wo matmul needs.
- Normalization (divide by softmax sum) happens on ctxT via reciprocal +
  a rank-1 PE broadcast matmul; it is emitted one head late so the PE always
  has the next head's score matmuls available while the reciprocal runs.
- Score tiles are paired two k-chunks per PSUM tile so one Exp activation
  (and one diagonal-mask multiply) covers both.
"""

import numpy as np
import ml_dtypes

import concourse.bass as bass  # noqa: F401
import concourse.mybir as mybir
import concourse.tile as tile
from concourse import bacc
from concourse.bass_utils import run_bass_kernel_spmd

B, S, D, H, DH = 2, 2048, 1024, 16, 64
N_CORES = 8
HPC = 4            # heads per core
DG = HPC * DH      # 256 head dims per core
QW = 512           # q-chunk width
NQ = S // QW       # 4 q-chunks
NKC = S // 128     # 16 k-chunks
NDC = D // 128     # 8 contraction chunks for projections

BF = mybir.dt.bfloat16
F32 = mybir.dt.float32
F32R = mybir.dt.float32r

_CACHE = {}


def _emit(nc):
    xqT = nc.dram_tensor("xqT", [D, S], BF, kind="ExternalInput")
    xkT = nc.dram_tensor("xkT", [D, S], BF, kind="ExternalInput")
    xvT = nc.dram_tensor("xvT", [D, S], BF, kind="ExternalInput")
    wqT = nc.dram_tensor("wqT", [D, DG], BF, kind="ExternalInput")
    wkT = nc.dram_tensor("wkT", [D, DG], BF, kind="ExternalInput")
    wvT = nc.dram_tensor("wvT", [D, DG], BF, kind="ExternalInput")
    woT = nc.dram_tensor("woT", [DG, D], BF, kind="ExternalInput")
    mskd = nc.dram_tensor("msk", [128, 4, QW], BF, kind="ExternalInput")
    onesd = nc.dram_tensor("ones", [1, 64], F32R, kind="ExternalInput")
    outp = nc.dram_tensor("outp", [S, D], BF, kind="ExternalOutput")

    EXP = mybir.ActivationFunctionType.Exp

    with tile.TileContext(nc) as tc:
        with (
            tc.tile_pool(name="wpool", bufs=1) as wpool,
            tc.tile_pool(name="spool", bufs=1) as spool,
            tc.tile_pool(name="xpool", bufs=2) as xpool,
            tc.tile_pool(name="apool", bufs=10) as apool,
            tc.tile_pool(name="cpool", bufs=6) as cpool,
            tc.tile_pool(name="rpool", bufs=4) as rpool,
            tc.tile_pool(name="opool", bufs=6) as opool,
            tc.tile_pool(name="ppair", bufs=2, space="PSUM") as ppair,
            tc.tile_pool(name="pmain", bufs=2, space="PSUM") as pmain,
            tc.tile_pool(name="pctx", bufs=2, space="PSUM") as pctx,
        ):
            # --- persistent tiles ---
            wq = wpool.tile([128, NDC, DG], BF)
            wk = wpool.tile([128, NDC, DG], BF)
            wv = wpool.tile([128, NDC, DG], BF)
            wo = wpool.tile([128, 2, D], BF)
            msk = wpool.tile([128, 4, QW], BF)
            ones = wpool.tile([1, 64], F32R)
            qT = spool.tile([128, 2, S], BF)
            kT = spool.tile([128, 2, S], BF)
            vv = spool.tile([128, NKC, HPC, DH + 1], BF)
            ctxT = spool.tile([128, 2, S], BF)

            nc.sync.dma_start(wq[:], wqT.ap().rearrange("(c p) n -> p c n", p=128))
            nc.vector.memset(vv[:, :, :, DH : DH + 1], 1.0)

            # --- input DMAs (chunked, alternating the two HWDGE queues) ---
            dma_engines = (nc.sync, nc.scalar)
            xq = xpool.tile([128, NDC, S], BF, tag="xt")
            xk = xpool.tile([128, NDC, S], BF, tag="xt")
            for c in range(NDC):
                dma_engines[c % 2].dma_start(
                    xq[:, c, :], xqT.ap()[c * 128 : (c + 1) * 128, :]
                )
            nc.scalar.dma_start(wk[:], wkT.ap().rearrange("(c p) n -> p c n", p=128))
            for c in range(NDC):
                dma_engines[c % 2].dma_start(
                    xk[:, c, :], xkT.ap()[c * 128 : (c + 1) * 128, :]
                )

            for w, xt, dst in ((wq, xq, qT), (wk, xk, kT)):
                for t in range(2):
                    for si in range(NQ):
                        ps = pmain.tile([128, QW], F32, tag="ps")
                        for c in range(NDC):
                            nc.tensor.matmul(
                                ps[:],
                                w[:, c, t * 128 : (t + 1) * 128],
                                xt[:, c, si * QW : (si + 1) * QW],
                                start=(c == 0),
                                stop=(c == NDC - 1),
                            )
                        nc.scalar.copy(dst[:, t, si * QW : (si + 1) * QW], ps[:])

            # late weight loads (not needed until v-proj / attention)
            nc.sync.dma_start(wv[:], wvT.ap().rearrange("(c p) n -> p c n", p=128))
            nc.sync.dma_start(msk[:], mskd.ap())
            nc.sync.dma_start(ones[:], onesd.ap())

            # --- projection: v (natural [s, dh]) ---
            xv = xpool.tile([128, NDC, S], BF, tag="xt")
            for c in range(NDC):
                dma_engines[c % 2].dma_start(
                    xv[:, c, :], xvT.ap()[c * 128 : (c + 1) * 128, :]
                )
            # wo is not needed until the first output projection (~60us in)
            nc.scalar.dma_start(wo[:], woT.ap().rearrange("(c p) n -> p c n", p=128))

            def emit_vproj(st):
                ps = pmain.tile([128, DG], F32, tag="ps")
                for c in range(NDC):
                    nc.tensor.matmul(
                        ps[:],
                        xv[:, c, st * 128 : (st + 1) * 128],
                        wv[:, c, :],
                        start=(c == 0),
                        stop=(c == NDC - 1),
                    )
                nc.vector.tensor_copy(
                    vv[:, st, :, 0:DH],
                    ps[:].rearrange("p (h e) -> p h e", e=DH),
                )

            # --- attention + output projection, per q-chunk ---
            def emit_norm_a(qi, h, cps):
                # stage A: reciprocal of the sums row + lift ctx out of PSUM
                rc = rpool.tile([1, QW], F32R)
                with nc.allow_low_precision(reason="f32r bits ~ f32"):
                    nc.vector.reciprocal(rc[:], cps[DH : DH + 1, :])
                cu = cpool.tile([64, QW], BF)
                nc.vector.tensor_copy(cu[:], cps[0:DH, :])
                return rc, cu

            def emit_norm_b(qi, h, rc, cu):
                # stage B: rank-1 broadcast of 1/sum and the normalize multiply
                t, p0 = h // 2, 64 * (h % 2)
                q_sl = slice(qi * QW, (qi + 1) * QW)
                bps = pmain.tile([64, QW], F32, tag="ps")
                nc.tensor.matmul(bps[:], ones[:], rc[:], start=True, stop=True)
                nc.vector.tensor_mul(ctxT[p0 : p0 + 64, t, q_sl], cu[:], bps[:])

            def emit_wo(qt):
                # 1-bank accumulators from pmain so Wo never occupies the
                # score-pair slots that feed the exp stream
                ob = opool.tile([128, D], BF)
                for nh in range(2):
                    ops = pmain.tile([128, 512], F32, tag="ps")
                    for t in range(2):
                        nc.tensor.matmul(
                            ops[:],
                            ctxT[:, t, qt * 128 : (qt + 1) * 128],
                            wo[:, t, nh * 512 : (nh + 1) * 512],
                            start=(t == 0),
                            stop=(t == 1),
                        )
                    nc.vector.tensor_copy(ob[:, nh * 512 : (nh + 1) * 512], ops[:])
                dma_engines[qt % 2].dma_start(
                    outp.ap()[qt * 128 : (qt + 1) * 128, :], ob[:]
                )

            for qi in range(NQ):
                # v-projection chunks needed by this q-chunk's attention
                for st in range(4 * qi, 4 * qi + 4):
                    emit_vproj(st)
                q_sl = slice(qi * QW, (qi + 1) * QW)
                nk = (qi + 1) * 4
                pending = None
                for h in range(HPC):
                    if qi > 0 and h >= 1:
                        emit_wo((qi - 1) * 4 + h - 1)
                    t, p0 = h // 2, 64 * (h % 2)
                    cps = pctx.tile([DH + 1, QW], F32, tag="cps")
                    for pc in range(nk // 2):
                        d0 = 2 * pc - qi * 4
                        # far diagonal pair (d0=2): columns [0:256) are fully
                        # causal-masked for both halves; compute half-width.
                        c0 = 256 if d0 == 2 else 0
                        csl = slice(c0, QW)
                        sps = ppair.tile([128, 2, QW], F32, tag="sps")
                        for half in range(2):
                            kc = 2 * pc + half
                            nc.tensor.matmul(
                                sps[:, half, csl],
                                kT[p0 : p0 + 64, t, kc * 128 : (kc + 1) * 128],
                                qT[p0 : p0 + 64, t, qi * QW + c0 : (qi + 1) * QW],
                                start=True,
                                stop=True,
                            )
                        at = apool.tile([128, 2, QW], BF, tag="at")
                        nc.scalar.activation(at[:, :, csl], sps[:, :, csl], EXP)
                        if d0 >= 0:
                            nc.vector.tensor_mul(
                                at[:, :, csl], at[:, :, csl], msk[:, d0 : d0 + 2, csl]
                            )
                        for half in range(2):
                            kc = 2 * pc + half
                            nc.tensor.matmul(
                                cps[:, csl],
                                vv[:, kc, h, :],
                                at[:, half, csl],
                                start=(kc == 0),
                                stop=(kc == nk - 1),
                            )
                    if pending is not None:
                        ph, prc, pcu = pending
                        emit_norm_b(qi, ph, prc, pcu)
                    rc, cu = emit_norm_a(qi, h, cps)
                    pending = (h, rc, cu)
                ph, prc, pcu = pending
                emit_norm_b(qi, ph, prc, pcu)
                if qi > 0:
                    emit_wo(qi * 4 - 1)
            for j in range(4):
                emit_wo(12 + j)


def build_program():
    if "nc" in _CACHE:
        return _CACHE["nc"]
    nc = bacc.Bacc(
        "TRN2", target_bir_lowering=False, debug=False, num_devices=N_CORES
    )
    _emit(nc)
    nc.compile()
    _CACHE["nc"] = nc
    return nc


def _prep_in_maps(query, key, value, Wq, Wk, Wv, Wo):
    bf = ml_dtypes.bfloat16
    scale = 1.0 / np.sqrt(np.float32(DH))

    p, i, j = np.ogrid[0:128, 0:4, 0:QW]
    msk = (j >= 128 * i + p).astype(bf)

    xT = {}
    for b in range(B):
        xT[("q", b)] = np.ascontiguousarray(query[b].T).astype(bf)
        xT[("k", b)] = np.ascontiguousarray(key[b].T).astype(bf)
        xT[("v", b)] = np.ascontiguousarray(value[b].T).astype(bf)

    in_maps = []
    for c in range(N_CORES):
        b, g = c // HPC, c % HPC
        rows = slice(g * DG, (g + 1) * DG)
        in_maps.append(
            {
                "xqT": xT[("q", b)],
                "xkT": xT[("k", b)],
                "xvT": xT[("v", b)],
                "wqT": np.ascontiguousarray((Wq[rows] * scale).T).astype(bf),
                "wkT": np.ascontiguousarray(Wk[rows].T).astype(bf),
                "wvT": np.ascontiguousarray(Wv[rows].T).astype(bf),
                "woT": np.ascontiguousarray(Wo[:, rows].T).astype(bf),
                "msk": msk,
                "ones": np.ones((1, 64), dtype=np.float32),
            }
        )
    return in_maps


def kernel(query, key, value, Wq, Wk, Wv, Wo):
    query = np.asarray(query, dtype=np.float32)
    key = np.asarray(key, dtype=np.float32)
    value = np.asarray(value, dtype=np.float32)
    Wq = np.asarray(Wq, dtype=np.float32)
    Wk = np.asarray(Wk, dtype=np.float32)
    Wv = np.asarray(Wv, dtype=np.float32)
    Wo = np.asarray(Wo, dtype=np.float32)

    nc = build_program()
    in_maps = _prep_in_maps(query, key, value, Wq, Wk, Wv, Wo)
    res = run_bass_kernel_spmd(
        nc, in_maps, core_ids=list(range(N_CORES)), trace=False
    )
    out = np.zeros((B, S, D), dtype=np.float32)
    for b in range(B):
        for g in range(HPC):
            out[b] += res.results[b * HPC + g]["outp"].astype(np.float32)
    return out


# revision 74
# speedup vs baseline: 1.0856x; 1.0856x over previous
"""Multi-head causal attention (B=2, S=2048, D=1024, H=16, DH=64) on 8 NeuronCores.

Sharding: data-parallel over batch (2) x tensor-parallel over heads (4 groups
of 4 heads). Core c handles batch c//4, heads 4*(c%4)..4*(c%4)+3. Each core
computes its head-group's Q/K/V projections, causal softmax attention, and a
partial output projection (Wo row-shard); the host sums the 4 partials per
batch.

Device-side layout choices:
- Inputs are uploaded transposed (xT: [D, S]) and in bf16 so every projection
  matmul contracts over the partition dim with contiguous DMA.
- q/k are produced transposed ([dh, s]); scores are computed transposed
  ([k, q]) so the softmax sum folds into the attention*V matmul via an extra
  ones-column on V, and the context comes out as ctxT [dh, q], which is
  exactly the stationary operand the Wo matmul needs.
- Normalization (divide by softmax sum) happens on ctxT via reciprocal +
  a rank-1 PE broadcast matmul; it is emitted one head late so the PE always
  has the next head's score matmuls available while the reciprocal runs.
- Score tiles are paired two k-chunks per PSUM tile so one Exp activation
  (and one diagonal-mask multiply) covers both.
"""

import numpy as np
import ml_dtypes

import concourse.bass as bass  # noqa: F401
import concourse.mybir as mybir
import concourse.tile as tile
from concourse import bacc
from concourse.bass_utils import run_bass_kernel_spmd

B, S, D, H, DH = 2, 2048, 1024, 16, 64
N_CORES = 8
HPC = 4            # heads per core
DG = HPC * DH      # 256 head dims per core
QW = 512           # q-chunk width
NQ = S // QW       # 4 q-chunks
NKC = S // 128     # 16 k-chunks
NDC = D // 128     # 8 contraction chunks for projections

BF = mybir.dt.bfloat16
F32 = mybir.dt.float32
F32R = mybir.dt.float32r

_CACHE = {}


def _emit(nc):
    xqT = nc.dram_tensor("xqT", [D, S], BF, kind="ExternalInput")
    xkT = nc.dram_tensor("xkT", [D, S], BF, kind="ExternalInput")
    xvT = nc.dram_tensor("xvT", [D, S], BF, kind="ExternalInput")
    wqT = nc.dram_tensor("wqT", [D, DG], BF, kind="ExternalInput")
    wkT = nc.dram_tensor("wkT", [D, DG], BF, kind="ExternalInput")
    wvT = nc.dram_tensor("wvT", [D, DG], BF, kind="ExternalInput")
    woT = nc.dram_tensor("woT", [DG, D], BF, kind="ExternalInput")
    mskd = nc.dram_tensor("msk", [128, 4, QW], BF, kind="ExternalInput")
    onesd = nc.dram_tensor("ones", [1, 64], F32R, kind="ExternalInput")
    outp = nc.dram_tensor("outp", [S, D], BF, kind="ExternalOutput")

    EXP = mybir.ActivationFunctionType.Exp

    with tile.TileContext(nc) as tc:
        with (
            tc.tile_pool(name="wpool", bufs=1) as wpool,
            tc.tile_pool(name="spool", bufs=1) as spool,
            tc.tile_pool(name="xpool", bufs=2) as xpool,
            tc.tile_pool(name="apool", bufs=10) as apool,
            tc.tile_pool(name="cpool", bufs=6) as cpool,
            tc.tile_pool(name="rpool", bufs=4) as rpool,
            tc.tile_pool(name="opool", bufs=6) as opool,
            tc.tile_pool(name="ppair", bufs=2, space="PSUM") as ppair,
            tc.tile_pool(name="pmain", bufs=2, space="PSUM") as pmain,
            tc.tile_pool(name="pctx", bufs=2, space="PSUM") as pctx,
        ):
            # --- persistent tiles ---
            wq = wpool.tile([128, NDC, DG], BF)
            wk = wpool.tile([128, NDC, DG], BF)
            wv = wpool.tile([128, NDC, DG], BF)
            wo = wpool.tile([128, 2, D], BF)
            msk = wpool.tile([128, 4, QW], BF)
            ones = wpool.tile([1, 64], F32R)
            qT = spool.tile([128, 2, S], BF)
            kT = spool.tile([128, 2, S], BF)
            vv = spool.tile([128, NKC, HPC, DH + 1], BF)
            ctxT = spool.tile([128, 2, S], BF)

            nc.sync.dma_start(wq[:], wqT.ap().rearrange("(c p) n -> p c n", p=128))
            nc.scalar.dma_start(wk[:], wkT.ap().rearrange("(c p) n -> p c n", p=128))
            nc.vector.memset(vv[:, :, :, DH : DH + 1], 1.0)

            # --- input DMAs (chunked, alternating the two HWDGE queues) ---
            dma_engines = (nc.sync, nc.scalar)
            xq = xpool.tile([128, NDC, S], BF, tag="xt")
            xk = xpool.tile([128, NDC, S], BF, tag="xt")
            for c in range(NDC):
                dma_engines[c % 2].dma_start(
                    xq[:, c, :], xqT.ap()[c * 128 : (c + 1) * 128, :]
                )
            for c in range(NDC):
                dma_engines[c % 2].dma_start(
                    xk[:, c, :], xkT.ap()[c * 128 : (c + 1) * 128, :]
                )

            for w, xt, dst in ((wq, xq, qT), (wk, xk, kT)):
                for t in range(2):
                    for si in range(NQ):
                        ps = pmain.tile([128, QW], F32, tag="ps")
                        for c in range(NDC):
                            nc.tensor.matmul(
                                ps[:],
                                w[:, c, t * 128 : (t + 1) * 128],
                                xt[:, c, si * QW : (si + 1) * QW],
                                start=(c == 0),
                                stop=(c == NDC - 1),
                            )
                        nc.scalar.copy(dst[:, t, si * QW : (si + 1) * QW], ps[:])

            # late weight loads (not needed until v-proj / attention)
            nc.sync.dma_start(wv[:], wvT.ap().rearrange("(c p) n -> p c n", p=128))
            nc.scalar.dma_start(wo[:], woT.ap().rearrange("(c p) n -> p c n", p=128))
            nc.sync.dma_start(msk[:], mskd.ap())
            nc.sync.dma_start(ones[:], onesd.ap())

            # --- projection: v (natural [s, dh]) ---
            xv = xpool.tile([128, NDC, S], BF, tag="xt")
            for c in range(NDC):
                dma_engines[c % 2].dma_start(
                    xv[:, c, :], xvT.ap()[c * 128 : (c + 1) * 128, :]
                )

            def emit_vproj(st):
                ps = pmain.tile([128, DG], F32, tag="ps")
                for c in range(NDC):
                    nc.tensor.matmul(
                        ps[:],
                        xv[:, c, st * 128 : (st + 1) * 128],
                        wv[:, c, :],
                        start=(c == 0),
                        stop=(c == NDC - 1),
                    )
                nc.vector.tensor_copy(
                    vv[:, st, :, 0:DH],
                    ps[:].rearrange("p (h e) -> p h e", e=DH),
                )

            # --- attention + output projection, per q-chunk ---
            def emit_norm_a(qi, h, cps):
                # stage A: reciprocal of the sums row + lift ctx out of PSUM
                rc = rpool.tile([1, QW], F32R)
                with nc.allow_low_precision(reason="f32r bits ~ f32"):
                    nc.vector.reciprocal(rc[:], cps[DH : DH + 1, :])
                cu = cpool.tile([64, QW], BF)
                nc.vector.tensor_copy(cu[:], cps[0:DH, :])
                return rc, cu

            def emit_norm_b(qi, h, rc, cu):
                # stage B: rank-1 broadcast of 1/sum and the normalize multiply
                t, p0 = h // 2, 64 * (h % 2)
                q_sl = slice(qi * QW, (qi + 1) * QW)
                bps = pmain.tile([64, QW], F32, tag="ps")
                nc.tensor.matmul(bps[:], ones[:], rc[:], start=True, stop=True)
                nc.vector.tensor_mul(ctxT[p0 : p0 + 64, t, q_sl], cu[:], bps[:])

            def emit_wo(qt):
                # 1-bank accumulators from pmain so Wo never occupies the
                # score-pair slots that feed the exp stream
                ob = opool.tile([128, D], BF)
                for nh in range(2):
                    ops = pmain.tile([128, 512], F32, tag="ps")
                    for t in range(2):
                        nc.tensor.matmul(
                            ops[:],
                            ctxT[:, t, qt * 128 : (qt + 1) * 128],
                            wo[:, t, nh * 512 : (nh + 1) * 512],
                            start=(t == 0),
                            stop=(t == 1),
                        )
                    nc.vector.tensor_copy(ob[:, nh * 512 : (nh + 1) * 512], ops[:])
                dma_engines[qt % 2].dma_start(
                    outp.ap()[qt * 128 : (qt + 1) * 128, :], ob[:]
                )

            for qi in range(NQ):
                # v-projection chunks needed by this q-chunk's attention
                for st in range(4 * qi, 4 * qi + 4):
                    emit_vproj(st)
                q_sl = slice(qi * QW, (qi + 1) * QW)
                nk = (qi + 1) * 4
                pending = None
                for h in range(HPC):
                    if qi > 0 and h >= 1:
                        emit_wo((qi - 1) * 4 + h - 1)
                    t, p0 = h // 2, 64 * (h % 2)
                    cps = pctx.tile([DH + 1, QW], F32, tag="cps")
                    for pc in range(nk // 2):
                        d0 = 2 * pc - qi * 4
                        # far diagonal pair (d0=2): columns [0:256) are fully
                        # causal-masked for both halves; compute half-width.
                        c0 = 256 if d0 == 2 else 0
                        csl = slice(c0, QW)
                        sps = ppair.tile([128, 2, QW], F32, tag="sps")
                        with tc.high_priority(offset=96):
                            for half in range(2):
                                kc = 2 * pc + half
                                nc.tensor.matmul(
                                    sps[:, half, csl],
                                    kT[p0 : p0 + 64, t, kc * 128 : (kc + 1) * 128],
                                    qT[p0 : p0 + 64, t, qi * QW + c0 : (qi + 1) * QW],
                                    start=True,
                                    stop=True,
                                )
                        at = apool.tile([128, 2, QW], BF, tag="at")
                        with tc.high_priority(offset=48):
                            nc.scalar.activation(at[:, :, csl], sps[:, :, csl], EXP)
                        if d0 >= 0:
                            nc.vector.tensor_mul(
                                at[:, :, csl], at[:, :, csl], msk[:, d0 : d0 + 2, csl]
                            )
                        for half in range(2):
                            kc = 2 * pc + half
                            nc.tensor.matmul(
                                cps[:, csl],
                                vv[:, kc, h, :],
                                at[:, half, csl],
                                start=(kc == 0),
                                stop=(kc == nk - 1),
                            )
                    if pending is not None:
                        ph, prc, pcu = pending
                        emit_norm_b(qi, ph, prc, pcu)
                    rc, cu = emit_norm_a(qi, h, cps)
                    pending = (h, rc, cu)
                ph, prc, pcu = pending
                emit_norm_b(qi, ph, prc, pcu)
                if qi > 0:
                    emit_wo(qi * 4 - 1)
            for j in range(4):
                emit_wo(12 + j)


def build_program():
    if "nc" in _CACHE:
        return _CACHE["nc"]
    nc = bacc.Bacc(
        "TRN2", target_bir_lowering=False, debug=False, num_devices=N_CORES
    )
    _emit(nc)
    nc.compile()
    _CACHE["nc"] = nc
    return nc


def _prep_in_maps(query, key, value, Wq, Wk, Wv, Wo):
    bf = ml_dtypes.bfloat16
    scale = 1.0 / np.sqrt(np.float32(DH))

    p, i, j = np.ogrid[0:128, 0:4, 0:QW]
    msk = (j >= 128 * i + p).astype(bf)

    xT = {}
    for b in range(B):
        xT[("q", b)] = np.ascontiguousarray(query[b].T).astype(bf)
        xT[("k", b)] = np.ascontiguousarray(key[b].T).astype(bf)
        xT[("v", b)] = np.ascontiguousarray(value[b].T).astype(bf)

    in_maps = []
    for c in range(N_CORES):
        b, g = c // HPC, c % HPC
        rows = slice(g * DG, (g + 1) * DG)
        in_maps.append(
            {
                "xqT": xT[("q", b)],
                "xkT": xT[("k", b)],
                "xvT": xT[("v", b)],
                "wqT": np.ascontiguousarray((Wq[rows] * scale).T).astype(bf),
                "wkT": np.ascontiguousarray(Wk[rows].T).astype(bf),
                "wvT": np.ascontiguousarray(Wv[rows].T).astype(bf),
                "woT": np.ascontiguousarray(Wo[:, rows].T).astype(bf),
                "msk": msk,
                "ones": np.ones((1, 64), dtype=np.float32),
            }
        )
    return in_maps


def kernel(query, key, value, Wq, Wk, Wv, Wo):
    query = np.asarray(query, dtype=np.float32)
    key = np.asarray(key, dtype=np.float32)
    value = np.asarray(value, dtype=np.float32)
    Wq = np.asarray(Wq, dtype=np.float32)
    Wk = np.asarray(Wk, dtype=np.float32)
    Wv = np.asarray(Wv, dtype=np.float32)
    Wo = np.asarray(Wo, dtype=np.float32)

    nc = build_program()
    in_maps = _prep_in_maps(query, key, value, Wq, Wk, Wv, Wo)
    res = run_bass_kernel_spmd(
        nc, in_maps, core_ids=list(range(N_CORES)), trace=False
    )
    out = np.zeros((B, S, D), dtype=np.float32)
    for b in range(B):
        for g in range(HPC):
            out[b] += res.results[b * HPC + g]["outp"].astype(np.float32)
    return out
